# revision 38
# baseline (speedup 1.0000x reference)
"""Trainium2 Bass kernel for nn_Attention_11458972746115 (v2).

Multi-head attention (B=1, S=2048, D=1024, H=16, DH=64) with RoPE and a
block-diagonal segment mask, tensor-parallel over heads across 8 NeuronCores
(2 heads per core).  Each core computes qkv projections, RoPE, block-sparse
masked attention and its slice of the output projection; the partial output
products (sum-sharded over the wo contraction) are reduced on the host.

v2 design (vs the f32r v1):
 - All data on the wire and in SBUF is bf16 (psum stays f32): halves DMA
   and enables 1-cycle/row matmuls at any moving size.
 - Both heads live together on 128 partitions end-to-end; RoPE pair-rotation
   is one pair of 128-wide permutation matmuls (isel/psel) per chunk.
 - No mask rows: attention windows are always fully inside a segment
   (unaligned 128-windows starting at each segment's lower bound), so the
   block-diagonal mask never actually masks anything we compute.
 - attn@v keeps the [vdim, q] orientation (F=512 moving) so ldweights stays
   hidden under the matmul stream; v is transposed per 128-window on the PE
   from a [vdim, seq] staging tile, and any projection chunk a window's
   transpose slice touches is emitted before that segment's attention
   (Tile treats emission-order RAW inversions as WAR, silently).
 - A ones column appended to each head's v block gives the softmax
   denominator as pot row 64; 1/r is computed directly from that psum row,
   broadcast across 64 partitions with a tiny C=1 ones matmul, and applied
   during the pot drain.
 - the two heads' renormalized outputs live in separate [64, S] tiles
   (DVE cannot write across partitions), so the output projection runs as
   two accumulating C=64 matmuls per tile against split wo halves.
 - Engine balance: PE does matmuls only; ACT does exp plus a share of the
   psum drains; DVE does the rope cos/sin muls, renorm and the rest of the
   drains (gpsimd cannot touch psum).  DMA queues are segregated so the
   in-order queues never cross-block: inputs on sync, outputs on the
   otherwise-idle gpsimd SWDGE ring - this is what lets iteration i+1's
   input loads overlap iteration i's attention/tail in the repeat loop.
"""

import os
import numpy as np

S = 2048
D = 1024
H = 16
DH = 64
NCORES = 8

_PROG_CACHE = {}


def _build(bounds, reps=1, debug=False):
    import contextlib

    import concourse.bacc as bacc
    import concourse.mybir as mybir
    import concourse.tile as tile
    from concourse.bass import ts

    f32 = mybir.dt.float32
    bf16 = mybir.dt.bfloat16
    AF = mybir.ActivationFunctionType

    segs = [(bounds[g], bounds[g + 1]) for g in range(4) if bounds[g + 1] > bounds[g]]

    allwins = []   # (w0, w1, anchor): window, plus 128-wide transpose anchor
    seg_wins = []
    for (lo, hi) in segs:
        idxs = []
        for w0 in range(lo, hi, 128):
            w1 = min(w0 + 128, hi)
            idxs.append(len(allwins))
            allwins.append((w0, w1, min(w0, S - 128)))
        seg_wins.append(idxs)
    NW = len(allwins)

    # chunks that must be projected before seg si's attention: its qa/ka
    # columns AND every window's 128-wide transpose slice (else the vga
    # drain would be emitted after its readers, which Tile treats as WAR,
    # not RAW - the cause of a first-run-only garbage bug)
    seg_need = []
    for si, (lo, hi) in enumerate(segs):
        need = (hi - 1) // 512
        for widx in seg_wins[si]:
            need = max(need, (allwins[widx][2] + 127) // 512)
        seg_need.append(min(3, need))

    # per-seg balanced score chunks of <=512 columns (the last segment is
    # split at least in two so its tail tiles can start earlier)
    def _chunks(lo, hi, nmin=1):
        n = max(-(-(hi - lo) // 512), nmin)
        base, rem = (hi - lo) // n, (hi - lo) % n
        out, p = [], lo
        for i in range(n):
            w = base + (1 if i < rem else 0)
            out.append((p, p + w))
            p += w
        return out
    seg_sc = [_chunks(lo, hi) for (lo, hi) in segs]
    # split the last segment's tail finely so the closing
    # renorm->tail chain is short
    llo, lhi = segs[-1]
    if lhi - llo > 384:
        cut = llo + (lhi - llo) - 256
        seg_sc[-1] = _chunks(llo, cut) + _chunks(cut, lhi, 2)

    nc = bacc.Bacc(None, target_bir_lowering=False)

    xts_d = nc.dram_tensor("xts", [128, 8 * S], bf16, kind="ExternalInput")
    wqs_d = nc.dram_tensor("wqs", [128, 1024], bf16, kind="ExternalInput")
    wks_d = nc.dram_tensor("wks", [128, 1024], bf16, kind="ExternalInput")
    wvs_d = nc.dram_tensor("wvs", [128, 1024], bf16, kind="ExternalInput")
    wos_d = nc.dram_tensor("wos", [128, 1024], bf16, kind="ExternalInput")
    cs_d = nc.dram_tensor("cs2", [128, S], bf16, kind="ExternalInput")
    sn_d = nc.dram_tensor("sn2", [128, S], bf16, kind="ExternalInput")
    sel_d = nc.dram_tensor("sels", [128, 256], bf16, kind="ExternalInput")
    id_d = nc.dram_tensor("idon", [128, 128], bf16, kind="ExternalInput")
    out_d = nc.dram_tensor("outp", [S, D], bf16, kind="ExternalOutput")
    if debug:
        dbg = {n: nc.dram_tensor(f"dbg_{n}", shp, bf16, kind="ExternalOutput")
               for n, shp in (("qa", [128, S]), ("ka", [128, S]),
                              ("vt", [128, S]), ("osb", [128, S]))}
        nwn = len(allwins)
        dbg["vga"] = nc.dram_tensor("dbg_vga", [128, nwn * 130], bf16,
                                    kind="ExternalOutput")

    with tile.TileContext(nc, linearize=bool(os.environ.get("KLIN"))) as tc:
        rep_ctx = (tc.For_i(0, reps, 1, hint_engines=(
            mybir.EngineType.PE, mybir.EngineType.DVE,
            mybir.EngineType.Activation, mybir.EngineType.SP,
            mybir.EngineType.Pool))
                   if reps > 1 else contextlib.nullcontext())
        with rep_ctx, \
             tc.tile_pool(name="pj", bufs=1, space="PSUM") as PJ, \
             tc.tile_pool(name="stp", bufs=3, space="PSUM") as STP, \
             tc.tile_pool(name="pop", bufs=3, space="PSUM") as POP, \
             tc.tile_pool(name="wp", bufs=1) as wp, \
             tc.tile_pool(name="xp", bufs=1) as xp, \
             tc.tile_pool(name="pers", bufs=1) as pers, \
             tc.tile_pool(name="abp", bufs=2) as abp, \
             tc.tile_pool(name="up", bufs=3) as up, \
             tc.tile_pool(name="ocp", bufs=3) as ocp:

            wq_sb = wp.tile([128, 1024], bf16)
            wk_sb = wp.tile([128, 1024], bf16)
            wv_sb = wp.tile([128, 1024], bf16)
            wo0_sb = wp.tile([64, 1024], bf16)
            wo1_sb = wp.tile([64, 1024], bf16)
            cs_sb = wp.tile([128, S], bf16)
            sn_sb = wp.tile([128, S], bf16)
            sel_sb = wp.tile([128, 256], bf16)
            id_sb = wp.tile([128, 128], bf16)
            for t, dt_ in ((wq_sb, wqs_d), (wk_sb, wks_d), (wv_sb, wvs_d),
                           (cs_sb, cs_d), (sn_sb, sn_d), (sel_sb, sel_d),
                           (id_sb, id_d)):
                nc.sync.dma_start(t[:], dt_[:])
            nc.sync.dma_start(wo0_sb[:], wos_d[0:64, :])
            nc.sync.dma_start(wo1_sb[:], wos_d[64:128, :])
            xt = [xp.tile([128, S], bf16, name=f"xt{d}") for d in range(8)]
            for half in range(2):
                for d in range(8):
                    h0 = half * 1024
                    nc.sync.dma_start(
                        xt[d][:, h0:h0 + 1024],
                        xts_d[:, d * S + h0:d * S + h0 + 1024])

            qa = pers.tile([128, S], bf16, name="qa")
            ka = pers.tile([128, S], bf16, name="ka")
            vt_sb = pers.tile([128, S], bf16, name="vt")
            vga = pers.tile([128, NW, 130], bf16, name="vga")
            osb0 = pers.tile([64, S], bf16, name="osb0")
            osb1 = pers.tile([64, S], bf16, name="osb1")
            rvs = [pers.tile([65, S], bf16, name=f"rv{h}") for h in (0, 1)]
            ones_sb = pers.tile([65, 64], bf16, name="ones")

            # ones column 64 of each head's 65-block in vga; ones rows for
            # the 1/r broadcast matmul
            ones_cols = vga.rearrange(
                "p w (a c) -> p w a c", a=2, c=65)[:, :, :, 64:65]
            nc.gpsimd.memset(ones_cols.bitcast(mybir.dt.uint16), 0x3F80)
            nc.gpsimd.memset(ones_sb[:].bitcast(mybir.dt.uint16), 0x3F80)

            npj = [0]

            def pj_tile(name):
                t = PJ.tile([128, 512], f32, tag=f"pj{npj[0] % 2}", name=name)
                npj[0] += 1
                return t

            def proj_chunk(ci):
                c0 = ci * 512
                qb = pj_tile(f"pq{ci}")
                for d in range(8):
                    nc.tensor.matmul(qb[:], wq_sb[:, ts(d, 128)],
                                     xt[d][:, c0:c0 + 512],
                                     start=(d == 0), stop=(d == 7))
                aq = abp.tile([128, 512], bf16, tag="aq", name=f"aq{ci}")
                bq = abp.tile([128, 512], bf16, tag="bq", name=f"bq{ci}")
                nc.vector.tensor_mul(aq[:], qb[:], cs_sb[:, c0:c0 + 512])
                nc.vector.tensor_mul(bq[:], qb[:], sn_sb[:, c0:c0 + 512])
                kb = pj_tile(f"pk{ci}")
                for d in range(8):
                    nc.tensor.matmul(kb[:], wk_sb[:, ts(d, 128)],
                                     xt[d][:, c0:c0 + 512],
                                     start=(d == 0), stop=(d == 7))
                ak = abp.tile([128, 512], bf16, tag="ak", name=f"ak{ci}")
                bk = abp.tile([128, 512], bf16, tag="bk", name=f"bk{ci}")
                nc.vector.tensor_mul(ak[:], kb[:], cs_sb[:, c0:c0 + 512])
                nc.vector.tensor_mul(bk[:], kb[:], sn_sb[:, c0:c0 + 512])
                rq = pj_tile(f"rq{ci}")
                nc.tensor.matmul(rq[:], sel_sb[:, 128:256], bq[:],
                                 start=True, stop=False)
                nc.tensor.matmul(rq[:], sel_sb[:, 0:128], aq[:],
                                 start=False, stop=True)
                nc.vector.tensor_copy(qa[:, c0:c0 + 512], rq[:])
                rk = pj_tile(f"rk{ci}")
                nc.tensor.matmul(rk[:], sel_sb[:, 128:256], bk[:],
                                 start=True, stop=False)
                nc.tensor.matmul(rk[:], sel_sb[:, 0:128], ak[:],
                                 start=False, stop=True)
                nc.vector.tensor_copy(ka[:, c0:c0 + 512], rk[:])
                vb = pj_tile(f"pv{ci}")
                for d in range(8):
                    nc.tensor.matmul(vb[:], wv_sb[:, ts(d, 128)],
                                     xt[d][:, c0:c0 + 512],
                                     start=(d == 0), stop=(d == 7))
                nc.scalar.copy(vt_sb[:, c0:c0 + 512], vb[:])

            def vt_window(widx):
                w0, w1, anchor = allwins[widx]
                wd = w1 - w0
                poff = w0 - anchor
                tp = STP.tile([128, 128], bf16, tag="st", name=f"tp{widx}")
                nc.tensor.transpose(tp[:], vt_sb[:, anchor:anchor + 128],
                                    id_sb[:])
                dst = vga[poff:poff + wd, widx, :].rearrange(
                    "p (a c) -> p a c", a=2, c=65)[:, :, 0:64]
                src = tp[poff:poff + wd, :].rearrange(
                    "p (a c) -> p a c", a=2, c=64)
                nc.vector.tensor_copy(dst, src)

            def attention_chunk(si, clo, chi):
                wins = seg_wins[si]
                cw = chi - clo
                plo, phi = clo, chi
                if cw % 2:
                    # pad left when possible: those columns are always
                    # already written (earlier chunks), never read-ahead
                    if clo > 0:
                        plo -= 1
                    else:
                        phi += 1
                cwp = phi - plo
                off = clo - plo

                for h in (0, 1):
                    pot = POP.tile([65, 512], f32, tag="po",
                                   name=f"po{si}_{h}")

                    def _av(ti, widx, u):
                        w0, w1, anchor = allwins[widx]
                        wd = w1 - w0
                        poff = w0 - anchor
                        nc.tensor.matmul(
                            pot[:, 0:cwp],
                            vga[poff:poff + wd, widx, ts(h, 65)],
                            u[0:wd, 0:cwp],
                            start=(ti == 0), stop=(ti == len(wins) - 1),
                            tile_position=(0, 0) if poff else None)

                    pend = None
                    for ti, widx in enumerate(wins):
                        w0, w1, anchor = allwins[widx]
                        wd = w1 - w0
                        stl = STP.tile([128, 512], f32, tag="st",
                                       name=f"st{si}_{h}")
                        nc.tensor.matmul(stl[0:wd, 0:cwp],
                                         ka[ts(h, 64), w0:w1],
                                         qa[ts(h, 64), plo:phi],
                                         start=True, stop=True)
                        if pend is not None:
                            _av(*pend)
                        u = up.tile([128, 512], bf16, tag="u", name="u")
                        nc.scalar.activation(u[0:wd, 0:cwp],
                                             stl[0:wd, 0:cwp], AF.Exp)
                        pend = (ti, widx, u)
                    _av(*pend)

                    # renorm: broadcast the fused denominator row across 64
                    # partitions with a C=1 matmul, take 1/r while draining
                    # it out of psum, and apply during the pot drain
                    nc.vector.tensor_copy(rvs[h][64:65, plo:phi],
                                          pot[64:65, 0:cwp])
                    rtv = STP.tile([64, 512], f32, tag="st", name="rtv")
                    nc.tensor.matmul(rtv[:, 0:cwp], ones_sb[64:65, :],
                                     rvs[h][64:65, plo:phi],
                                     start=True, stop=True)
                    rv = up.tile([64, 512], f32, tag="rv", name="rv")
                    nc.vector.reciprocal_approx_fast(rv[:, 0:cwp],
                                                     rtv[:, 0:cwp])
                    dst = osb0 if h == 0 else osb1
                    nc.vector.tensor_mul(dst[:, clo:chi],
                                         pot[0:64, off:off + cw],
                                         rv[:, off:off + cw])

            noc = [0]

            def tail_tile(i):
                oc = ocp.tile([128, 1024], bf16, tag="oc", name=f"oc{i}")
                for jj in (0, 1):
                    po = pj_tile(f"po{i}_{jj}")
                    nc.tensor.matmul(po[:], osb0[:, ts(i, 128)],
                                     wo0_sb[:, ts(jj, 512)],
                                     start=True, stop=False)
                    nc.tensor.matmul(po[:], osb1[:, ts(i, 128)],
                                     wo1_sb[:, ts(jj, 512)],
                                     start=False, stop=True)
                    if noc[0] % 2:
                        nc.scalar.copy(oc[:, ts(jj, 512)], po[:])
                    else:
                        nc.vector.tensor_copy(oc[:, ts(jj, 512)], po[:])
                    noc[0] += 1
                # out DMAs ride the gpsimd SWDGE queue: nothing on-chip
                # reads them, and sync/scalar queues stay unblocked
                nc.gpsimd.dma_start(out_d[ts(i, 128), :], oc[:])

            emitted_c = 0
            done_t = 0
            done = 0
            pending_lim = [0]
            for si, (lo, hi) in enumerate(segs):
                ci_need = seg_need[si]
                while emitted_c <= ci_need:
                    proj_chunk(emitted_c)
                    emitted_c += 1
                    cov = emitted_c * 512
                    while done_t < NW and allwins[done_t][2] + 128 <= cov:
                        vt_window(done_t)
                        done_t += 1
                last = si == len(segs) - 1
                for (clo, chi) in seg_sc[si]:
                    # lagged tails: emit tiles made ready by the PREVIOUS
                    # chunk now, so they queue behind this chunk's matmuls
                    # instead of stalling PE on the renorm chain
                    while done < pending_lim[0]:
                        tail_tile(done)
                        done += 1
                    attention_chunk(si, clo, chi)
                    pending_lim[0] = 16 if (last and chi == hi) else chi // 128
            while done < pending_lim[0]:
                tail_tile(done)
                done += 1

            if debug:
                for n, t in (("qa", qa), ("ka", ka), ("vt", vt_sb)):
                    nc.sync.dma_start(dbg[n][:], t[:])
                nc.sync.dma_start(dbg["osb"][0:64, :], osb0[:])
                nc.sync.dma_start(dbg["osb"][64:128, :], osb1[:])
                nc.sync.dma_start(
                    dbg["vga"][:],
                    vga.rearrange("p w c -> p (w c)"))

    nc.compile()
    return nc


def _host_tensors(x, seg, fc, fs, wq, wk, wv, wo):
    import ml_dtypes
    bf16 = ml_dtypes.bfloat16

    c64 = np.repeat(fc.T, 2, axis=0)
    s64 = np.empty((64, S), np.float32)
    s64[0::2] = fs.T
    s64[1::2] = -fs.T
    cos2 = np.ascontiguousarray(np.tile(c64, (2, 1))).astype(bf16)
    sin2 = np.ascontiguousarray(np.tile(s64, (2, 1))).astype(bf16)

    sel = np.zeros((128, 256), np.float32)
    sel[np.arange(128), np.arange(128)] = 1.0           # isel = I
    sel[np.arange(128) ^ 1, 128 + np.arange(128)] = 1.0  # psel[p^1, p]
    sels = sel.astype(bf16)

    idon = np.eye(128, dtype=np.float32).astype(bf16)

    xts = np.ascontiguousarray(
        x.T.reshape(8, 128, S).transpose(1, 0, 2)).reshape(128, 8 * S)
    xts = xts.astype(bf16)

    def wstack(w, scale):
        out = []
        for m in range(NCORES):
            wl = (w[m * 128:(m + 1) * 128, :] * scale).T.astype(np.float32)
            out.append(np.ascontiguousarray(
                wl.reshape(8, 128, 128).transpose(1, 0, 2)).reshape(
                    128, 1024).astype(bf16))
        return out

    wqs = wstack(wq, 0.125)
    wks = wstack(wk, 1.0)
    wvs = wstack(wv, 1.0)
    wos = [np.ascontiguousarray(wo[:, m * 128:(m + 1) * 128].T).astype(bf16)
           for m in range(NCORES)]

    common = {"xts": xts, "cs2": cos2, "sn2": sin2, "sels": sels,
              "idon": idon}
    in_maps = []
    for m in range(NCORES):
        im = dict(common)
        im["wqs"] = wqs[m]
        im["wks"] = wks[m]
        im["wvs"] = wvs[m]
        im["wos"] = wos[m]
        in_maps.append(im)
    return in_maps


def kernel(x, seg_ids, freqs_cos, freqs_sin, wq, wk, wv, wo):
    x = np.asarray(x, np.float32).reshape(S, D)
    seg = np.asarray(seg_ids).astype(np.int64)
    fc = np.asarray(freqs_cos, np.float32)
    fs = np.asarray(freqs_sin, np.float32)
    wq = np.asarray(wq, np.float32)
    wk = np.asarray(wk, np.float32)
    wv = np.asarray(wv, np.float32)
    wo = np.asarray(wo, np.float32)

    bounds = tuple(int(b) for b in np.searchsorted(seg, np.arange(5)))
    if bounds not in _PROG_CACHE:
        _PROG_CACHE[bounds] = _build(bounds)
    nc = _PROG_CACHE[bounds]

    in_maps = _host_tensors(x, seg, fc, fs, wq, wk, wv, wo)

    from concourse.bass_utils import run_bass_kernel_spmd

    trace = bool(os.environ.get("BASS_KERNEL_TRACE"))
    res = run_bass_kernel_spmd(nc, in_maps, core_ids=list(range(NCORES)),
                               trace=trace)
    if trace and res.exec_time_ns is not None:
        print(f"HW exec time: {res.exec_time_ns} ns")
        if res.instructions_and_trace is not None:
            print("trace:", res.instructions_and_trace[1])

    out = np.zeros((S, D), np.float32)
    for r in res.results:
        out += np.asarray(r["outp"], dtype=np.float32)
    return out.reshape(1, S, D)


# revision 39
# speedup vs baseline: 1.2593x; 1.2593x over previous
"""Trainium2 Bass kernel for nn_Attention_11458972746115 (v2).

Multi-head attention (B=1, S=2048, D=1024, H=16, DH=64) with RoPE and a
block-diagonal segment mask, tensor-parallel over heads across 8 NeuronCores
(2 heads per core).  Each core computes qkv projections, RoPE, block-sparse
masked attention and its slice of the output projection; the partial output
products (sum-sharded over the wo contraction) are reduced on the host.

v2 design (vs the f32r v1):
 - All data on the wire and in SBUF is bf16 (psum stays f32): halves DMA
   and enables 1-cycle/row matmuls at any moving size.
 - Both heads live together on 128 partitions end-to-end; RoPE pair-rotation
   is one pair of 128-wide permutation matmuls (isel/psel) per chunk.
 - No mask rows: attention windows are always fully inside a segment
   (unaligned 128-windows starting at each segment's lower bound), so the
   block-diagonal mask never actually masks anything we compute.
 - attn@v keeps the [vdim, q] orientation (F=512 moving) so ldweights stays
   hidden under the matmul stream; v is transposed per 128-window on the PE
   from a [vdim, seq] staging tile, and any projection chunk a window's
   transpose slice touches is emitted before that segment's attention
   (Tile treats emission-order RAW inversions as WAR, silently).
 - A ones column appended to each head's v block gives the softmax
   denominator as pot row 64; 1/r is computed directly from that psum row,
   broadcast across 64 partitions with a tiny C=1 ones matmul, and applied
   during the pot drain.
 - the two heads' renormalized outputs live in separate [64, S] tiles
   (DVE cannot write across partitions), so the output projection runs as
   two accumulating C=64 matmuls per tile against split wo halves.
 - Engine balance: PE does matmuls only; ACT does exp plus a share of the
   psum drains; DVE does the rope cos/sin muls, renorm and the rest of the
   drains (gpsimd cannot touch psum).  DMA queues are segregated so the
   in-order queues never cross-block: inputs on sync, outputs on the
   otherwise-idle gpsimd SWDGE ring - this is what lets iteration i+1's
   input loads overlap iteration i's attention/tail in the repeat loop.
"""

import os
import numpy as np

S = 2048
D = 1024
H = 16
DH = 64
NCORES = 8

_PROG_CACHE = {}


def _build(bounds, reps=1, debug=False):
    import contextlib

    import concourse.bacc as bacc
    import concourse.mybir as mybir
    import concourse.tile as tile
    from concourse.bass import ts

    f32 = mybir.dt.float32
    bf16 = mybir.dt.bfloat16
    AF = mybir.ActivationFunctionType

    segs = [(bounds[g], bounds[g + 1]) for g in range(4) if bounds[g + 1] > bounds[g]]

    allwins = []   # (w0, w1, anchor): window, plus 128-wide transpose anchor
    seg_wins = []
    for (lo, hi) in segs:
        idxs = []
        for w0 in range(lo, hi, 128):
            w1 = min(w0 + 128, hi)
            idxs.append(len(allwins))
            allwins.append((w0, w1, min(w0, S - 128)))
        seg_wins.append(idxs)
    NW = len(allwins)

    # chunks that must be projected before seg si's attention: its qa/ka
    # columns AND every window's 128-wide transpose slice (else the vga
    # drain would be emitted after its readers, which Tile treats as WAR,
    # not RAW - the cause of a first-run-only garbage bug)
    seg_need = []
    for si, (lo, hi) in enumerate(segs):
        need = (hi - 1) // 512
        for widx in seg_wins[si]:
            need = max(need, (allwins[widx][2] + 127) // 512)
        seg_need.append(min(3, need))

    # per-seg balanced score chunks of <=512 columns (the last segment is
    # split at least in two so its tail tiles can start earlier)
    def _chunks(lo, hi, nmin=1):
        n = max(-(-(hi - lo) // 512), nmin)
        base, rem = (hi - lo) // n, (hi - lo) % n
        out, p = [], lo
        for i in range(n):
            w = base + (1 if i < rem else 0)
            out.append((p, p + w))
            p += w
        return out
    seg_sc = [_chunks(lo, hi) for (lo, hi) in segs]
    # split the last segment's tail finely so the closing
    # renorm->tail chain is short
    llo, lhi = segs[-1]
    if lhi - llo > 384:
        cut = llo + (lhi - llo) - 256
        seg_sc[-1] = _chunks(llo, cut) + _chunks(cut, lhi, 2)

    nc = bacc.Bacc(None, target_bir_lowering=False)

    xts_d = nc.dram_tensor("xts", [128, 8 * S], bf16, kind="ExternalInput")
    wqs_d = nc.dram_tensor("wqs", [128, 1024], bf16, kind="ExternalInput")
    wks_d = nc.dram_tensor("wks", [128, 1024], bf16, kind="ExternalInput")
    wvs_d = nc.dram_tensor("wvs", [128, 1024], bf16, kind="ExternalInput")
    wos_d = nc.dram_tensor("wos", [128, 1024], bf16, kind="ExternalInput")
    cs_d = nc.dram_tensor("cs2", [128, S], bf16, kind="ExternalInput")
    sn_d = nc.dram_tensor("sn2", [128, S], bf16, kind="ExternalInput")
    sel_d = nc.dram_tensor("sels", [128, 256], bf16, kind="ExternalInput")
    id_d = nc.dram_tensor("idon", [128, 128], bf16, kind="ExternalInput")
    out_d = nc.dram_tensor("outp", [S, D], bf16, kind="ExternalOutput")
    if debug:
        dbg = {n: nc.dram_tensor(f"dbg_{n}", shp, bf16, kind="ExternalOutput")
               for n, shp in (("qa", [128, S]), ("ka", [128, S]),
                              ("vt", [128, S]), ("osb", [128, S]))}
        nwn = len(allwins)
        dbg["vga"] = nc.dram_tensor("dbg_vga", [128, nwn * 130], bf16,
                                    kind="ExternalOutput")

    with tile.TileContext(nc, linearize=bool(os.environ.get("KLIN"))) as tc:
        rep_ctx = (tc.For_i(0, reps, 1, hint_engines=(
            mybir.EngineType.PE, mybir.EngineType.DVE,
            mybir.EngineType.Activation, mybir.EngineType.SP,
            mybir.EngineType.Pool))
                   if reps > 1 else contextlib.nullcontext())
        with rep_ctx, \
             tc.tile_pool(name="pj", bufs=1, space="PSUM") as PJ, \
             tc.tile_pool(name="stp", bufs=3, space="PSUM") as STP, \
             tc.tile_pool(name="pop", bufs=3, space="PSUM") as POP, \
             tc.tile_pool(name="wp", bufs=1) as wp, \
             tc.tile_pool(name="xp", bufs=1) as xp, \
             tc.tile_pool(name="pers", bufs=1) as pers, \
             tc.tile_pool(name="abp", bufs=3) as abp, \
             tc.tile_pool(name="up", bufs=4) as up, \
             tc.tile_pool(name="ocp", bufs=4) as ocp:

            wq_sb = wp.tile([128, 1024], bf16)
            wk_sb = wp.tile([128, 1024], bf16)
            wv_sb = wp.tile([128, 1024], bf16)
            wo0_sb = wp.tile([64, 1024], bf16)
            wo1_sb = wp.tile([64, 1024], bf16)
            cs_sb = wp.tile([128, S], bf16)
            sn_sb = wp.tile([128, S], bf16)
            sel_sb = wp.tile([128, 256], bf16)
            id_sb = wp.tile([128, 128], bf16)
            for t, dt_ in ((wq_sb, wqs_d), (wk_sb, wks_d), (wv_sb, wvs_d),
                           (cs_sb, cs_d), (sn_sb, sn_d), (sel_sb, sel_d),
                           (id_sb, id_d)):
                nc.sync.dma_start(t[:], dt_[:])
            nc.sync.dma_start(wo0_sb[:], wos_d[0:64, :])
            nc.sync.dma_start(wo1_sb[:], wos_d[64:128, :])
            xt = [xp.tile([128, S], bf16, name=f"xt{d}") for d in range(8)]
            for half in range(2):
                for d in range(8):
                    h0 = half * 1024
                    nc.sync.dma_start(
                        xt[d][:, h0:h0 + 1024],
                        xts_d[:, d * S + h0:d * S + h0 + 1024])

            qa = pers.tile([128, S], bf16, name="qa")
            ka = pers.tile([128, S], bf16, name="ka")
            vt_sb = pers.tile([128, S], bf16, name="vt")
            vga = pers.tile([128, NW, 130], bf16, name="vga")
            osb0 = pers.tile([64, S], bf16, name="osb0")
            osb1 = pers.tile([64, S], bf16, name="osb1")
            rvs = [pers.tile([65, S], bf16, name=f"rv{h}") for h in (0, 1)]
            ones_sb = pers.tile([65, 64], bf16, name="ones")

            # ones column 64 of each head's 65-block in vga; ones rows for
            # the 1/r broadcast matmul
            ones_cols = vga.rearrange(
                "p w (a c) -> p w a c", a=2, c=65)[:, :, :, 64:65]
            nc.gpsimd.memset(ones_cols.bitcast(mybir.dt.uint16), 0x3F80)
            nc.gpsimd.memset(ones_sb[:].bitcast(mybir.dt.uint16), 0x3F80)

            npj = [0]

            def pj_tile(name):
                t = PJ.tile([128, 512], f32, tag=f"pj{npj[0] % 2}", name=name)
                npj[0] += 1
                return t

            def proj_chunk(ci):
                c0 = ci * 512
                qb = pj_tile(f"pq{ci}")
                for d in range(8):
                    nc.tensor.matmul(qb[:], wq_sb[:, ts(d, 128)],
                                     xt[d][:, c0:c0 + 512],
                                     start=(d == 0), stop=(d == 7))
                aq = abp.tile([128, 512], bf16, tag="aq", name=f"aq{ci}")
                bq = abp.tile([128, 512], bf16, tag="bq", name=f"bq{ci}")
                nc.vector.tensor_mul(aq[:], qb[:], cs_sb[:, c0:c0 + 512])
                nc.vector.tensor_mul(bq[:], qb[:], sn_sb[:, c0:c0 + 512])
                kb = pj_tile(f"pk{ci}")
                for d in range(8):
                    nc.tensor.matmul(kb[:], wk_sb[:, ts(d, 128)],
                                     xt[d][:, c0:c0 + 512],
                                     start=(d == 0), stop=(d == 7))
                ak = abp.tile([128, 512], bf16, tag="ak", name=f"ak{ci}")
                bk = abp.tile([128, 512], bf16, tag="bk", name=f"bk{ci}")
                nc.vector.tensor_mul(ak[:], kb[:], cs_sb[:, c0:c0 + 512])
                nc.vector.tensor_mul(bk[:], kb[:], sn_sb[:, c0:c0 + 512])
                rq = pj_tile(f"rq{ci}")
                nc.tensor.matmul(rq[:], sel_sb[:, 128:256], bq[:],
                                 start=True, stop=False)
                nc.tensor.matmul(rq[:], sel_sb[:, 0:128], aq[:],
                                 start=False, stop=True)
                nc.vector.tensor_copy(qa[:, c0:c0 + 512], rq[:])
                rk = pj_tile(f"rk{ci}")
                nc.tensor.matmul(rk[:], sel_sb[:, 128:256], bk[:],
                                 start=True, stop=False)
                nc.tensor.matmul(rk[:], sel_sb[:, 0:128], ak[:],
                                 start=False, stop=True)
                nc.vector.tensor_copy(ka[:, c0:c0 + 512], rk[:])
                vb = pj_tile(f"pv{ci}")
                for d in range(8):
                    nc.tensor.matmul(vb[:], wv_sb[:, ts(d, 128)],
                                     xt[d][:, c0:c0 + 512],
                                     start=(d == 0), stop=(d == 7))
                nc.scalar.copy(vt_sb[:, c0:c0 + 512], vb[:])

            def vt_window(widx):
                w0, w1, anchor = allwins[widx]
                wd = w1 - w0
                poff = w0 - anchor
                tp = STP.tile([128, 128], bf16, tag="st", name=f"tp{widx}")
                nc.tensor.transpose(tp[:], vt_sb[:, anchor:anchor + 128],
                                    id_sb[:])
                dst = vga[poff:poff + wd, widx, :].rearrange(
                    "p (a c) -> p a c", a=2, c=65)[:, :, 0:64]
                src = tp[poff:poff + wd, :].rearrange(
                    "p (a c) -> p a c", a=2, c=64)
                nc.vector.tensor_copy(dst, src)

            def attention_chunk(si, clo, chi):
                wins = seg_wins[si]
                cw = chi - clo
                plo, phi = clo, chi
                if cw % 2:
                    # pad left when possible: those columns are always
                    # already written (earlier chunks), never read-ahead
                    if clo > 0:
                        plo -= 1
                    else:
                        phi += 1
                cwp = phi - plo
                off = clo - plo

                for h in (0, 1):
                    pot = POP.tile([65, 512], f32, tag="po",
                                   name=f"po{si}_{h}")

                    def _av(ti, widx, u):
                        w0, w1, anchor = allwins[widx]
                        wd = w1 - w0
                        poff = w0 - anchor
                        nc.tensor.matmul(
                            pot[:, 0:cwp],
                            vga[poff:poff + wd, widx, ts(h, 65)],
                            u[0:wd, 0:cwp],
                            start=(ti == 0), stop=(ti == len(wins) - 1),
                            tile_position=(0, 0) if poff else None)

                    pend = None
                    for ti, widx in enumerate(wins):
                        w0, w1, anchor = allwins[widx]
                        wd = w1 - w0
                        stl = STP.tile([128, 512], f32, tag="st",
                                       name=f"st{si}_{h}")
                        nc.tensor.matmul(stl[0:wd, 0:cwp],
                                         ka[ts(h, 64), w0:w1],
                                         qa[ts(h, 64), plo:phi],
                                         start=True, stop=True)
                        if pend is not None:
                            _av(*pend)
                        u = up.tile([128, 512], bf16, tag="u", name="u")
                        nc.scalar.activation(u[0:wd, 0:cwp],
                                             stl[0:wd, 0:cwp], AF.Exp)
                        pend = (ti, widx, u)
                    _av(*pend)

                    # renorm: broadcast the fused denominator row across 64
                    # partitions with a C=1 matmul, take 1/r while draining
                    # it out of psum, and apply during the pot drain
                    nc.vector.tensor_copy(rvs[h][64:65, plo:phi],
                                          pot[64:65, 0:cwp])
                    rtv = STP.tile([64, 512], f32, tag="st", name="rtv")
                    nc.tensor.matmul(rtv[:, 0:cwp], ones_sb[64:65, :],
                                     rvs[h][64:65, plo:phi],
                                     start=True, stop=True)
                    rv = up.tile([64, 512], f32, tag="rv", name="rv")
                    nc.vector.reciprocal_approx_fast(rv[:, 0:cwp],
                                                     rtv[:, 0:cwp])
                    dst = osb0 if h == 0 else osb1
                    nc.vector.tensor_mul(dst[:, clo:chi],
                                         pot[0:64, off:off + cw],
                                         rv[:, off:off + cw])

            noc = [0]

            def tail_tile(i):
                oc = ocp.tile([128, 1024], bf16, tag="oc", name=f"oc{i}")
                for jj in (0, 1):
                    po = pj_tile(f"po{i}_{jj}")
                    nc.tensor.matmul(po[:], osb0[:, ts(i, 128)],
                                     wo0_sb[:, ts(jj, 512)],
                                     start=True, stop=False)
                    nc.tensor.matmul(po[:], osb1[:, ts(i, 128)],
                                     wo1_sb[:, ts(jj, 512)],
                                     start=False, stop=True)
                    if noc[0] % 2:
                        nc.scalar.copy(oc[:, ts(jj, 512)], po[:])
                    else:
                        nc.vector.tensor_copy(oc[:, ts(jj, 512)], po[:])
                    noc[0] += 1
                # out DMAs ride the gpsimd SWDGE queue: nothing on-chip
                # reads them, and sync/scalar queues stay unblocked
                nc.gpsimd.dma_start(out_d[ts(i, 128), :], oc[:])

            emitted_c = 0
            done_t = 0
            done = 0
            pending_lim = [0]
            for si, (lo, hi) in enumerate(segs):
                ci_need = seg_need[si]
                while emitted_c <= ci_need:
                    proj_chunk(emitted_c)
                    emitted_c += 1
                    cov = emitted_c * 512
                    while done_t < NW and allwins[done_t][2] + 128 <= cov:
                        vt_window(done_t)
                        done_t += 1
                last = si == len(segs) - 1
                for (clo, chi) in seg_sc[si]:
                    # lagged tails: emit tiles made ready by the PREVIOUS
                    # chunk now, so they queue behind this chunk's matmuls
                    # instead of stalling PE on the renorm chain
                    while done < pending_lim[0]:
                        tail_tile(done)
                        done += 1
                    attention_chunk(si, clo, chi)
                    pending_lim[0] = 16 if (last and chi == hi) else chi // 128
            while done < pending_lim[0]:
                tail_tile(done)
                done += 1

            if debug:
                for n, t in (("qa", qa), ("ka", ka), ("vt", vt_sb)):
                    nc.sync.dma_start(dbg[n][:], t[:])
                nc.sync.dma_start(dbg["osb"][0:64, :], osb0[:])
                nc.sync.dma_start(dbg["osb"][64:128, :], osb1[:])
                nc.sync.dma_start(
                    dbg["vga"][:],
                    vga.rearrange("p w c -> p (w c)"))

    nc.compile()
    return nc


def _host_tensors(x, seg, fc, fs, wq, wk, wv, wo):
    import ml_dtypes
    bf16 = ml_dtypes.bfloat16

    c64 = np.repeat(fc.T, 2, axis=0)
    s64 = np.empty((64, S), np.float32)
    s64[0::2] = fs.T
    s64[1::2] = -fs.T
    cos2 = np.ascontiguousarray(np.tile(c64, (2, 1))).astype(bf16)
    sin2 = np.ascontiguousarray(np.tile(s64, (2, 1))).astype(bf16)

    sel = np.zeros((128, 256), np.float32)
    sel[np.arange(128), np.arange(128)] = 1.0           # isel = I
    sel[np.arange(128) ^ 1, 128 + np.arange(128)] = 1.0  # psel[p^1, p]
    sels = sel.astype(bf16)

    idon = np.eye(128, dtype=np.float32).astype(bf16)

    xts = np.ascontiguousarray(
        x.T.reshape(8, 128, S).transpose(1, 0, 2)).reshape(128, 8 * S)
    xts = xts.astype(bf16)

    def wstack(w, scale):
        out = []
        for m in range(NCORES):
            wl = (w[m * 128:(m + 1) * 128, :] * scale).T.astype(np.float32)
            out.append(np.ascontiguousarray(
                wl.reshape(8, 128, 128).transpose(1, 0, 2)).reshape(
                    128, 1024).astype(bf16))
        return out

    wqs = wstack(wq, 0.125)
    wks = wstack(wk, 1.0)
    wvs = wstack(wv, 1.0)
    wos = [np.ascontiguousarray(wo[:, m * 128:(m + 1) * 128].T).astype(bf16)
           for m in range(NCORES)]

    common = {"xts": xts, "cs2": cos2, "sn2": sin2, "sels": sels,
              "idon": idon}
    in_maps = []
    for m in range(NCORES):
        im = dict(common)
        im["wqs"] = wqs[m]
        im["wks"] = wks[m]
        im["wvs"] = wvs[m]
        im["wos"] = wos[m]
        in_maps.append(im)
    return in_maps


def kernel(x, seg_ids, freqs_cos, freqs_sin, wq, wk, wv, wo):
    x = np.asarray(x, np.float32).reshape(S, D)
    seg = np.asarray(seg_ids).astype(np.int64)
    fc = np.asarray(freqs_cos, np.float32)
    fs = np.asarray(freqs_sin, np.float32)
    wq = np.asarray(wq, np.float32)
    wk = np.asarray(wk, np.float32)
    wv = np.asarray(wv, np.float32)
    wo = np.asarray(wo, np.float32)

    bounds = tuple(int(b) for b in np.searchsorted(seg, np.arange(5)))
    if bounds not in _PROG_CACHE:
        _PROG_CACHE[bounds] = _build(bounds)
    nc = _PROG_CACHE[bounds]

    in_maps = _host_tensors(x, seg, fc, fs, wq, wk, wv, wo)

    from concourse.bass_utils import run_bass_kernel_spmd

    trace = bool(os.environ.get("BASS_KERNEL_TRACE"))
    res = run_bass_kernel_spmd(nc, in_maps, core_ids=list(range(NCORES)),
                               trace=trace)
    if trace and res.exec_time_ns is not None:
        print(f"HW exec time: {res.exec_time_ns} ns")
        if res.instructions_and_trace is not None:
            print("trace:", res.instructions_and_trace[1])

    out = np.zeros((S, D), np.float32)
    for r in res.results:
        out += np.asarray(r["outp"], dtype=np.float32)
    return out.reshape(1, S, D)


# revision 40
# speedup vs baseline: 2.5499x; 2.0248x over previous
"""Trainium2 Bass kernel for nn_Attention_11458972746115 (v2).

Multi-head attention (B=1, S=2048, D=1024, H=16, DH=64) with RoPE and a
block-diagonal segment mask, tensor-parallel over heads across 8 NeuronCores
(2 heads per core).  Each core computes qkv projections, RoPE, block-sparse
masked attention and its slice of the output projection; the partial output
products (sum-sharded over the wo contraction) are reduced on the host.

v2 design (vs the f32r v1):
 - All data on the wire and in SBUF is bf16 (psum stays f32): halves DMA
   and enables 1-cycle/row matmuls at any moving size.
 - Both heads live together on 128 partitions end-to-end; RoPE pair-rotation
   is one pair of 128-wide permutation matmuls (isel/psel) per chunk.
 - No mask rows: attention windows are always fully inside a segment
   (unaligned 128-windows starting at each segment's lower bound), so the
   block-diagonal mask never actually masks anything we compute.
 - attn@v keeps the [vdim, q] orientation (F=512 moving) so ldweights stays
   hidden under the matmul stream; v is transposed per 128-window on the PE
   from a [vdim, seq] staging tile, and any projection chunk a window's
   transpose slice touches is emitted before that segment's attention
   (Tile treats emission-order RAW inversions as WAR, silently).
 - A ones column appended to each head's v block gives the softmax
   denominator as pot row 64; 1/r is computed directly from that psum row,
   broadcast across 64 partitions with a tiny C=1 ones matmul, and applied
   during the pot drain.
 - the two heads' renormalized outputs live in separate [64, S] tiles
   (DVE cannot write across partitions), so the output projection runs as
   two accumulating C=64 matmuls per tile against split wo halves.
 - Engine balance: PE does matmuls only; ACT does exp plus a share of the
   psum drains; DVE does the rope cos/sin muls, renorm and the rest of the
   drains (gpsimd cannot touch psum).  DMA queues are segregated so the
   in-order queues never cross-block: inputs on sync, outputs on the
   otherwise-idle gpsimd SWDGE ring - this is what lets iteration i+1's
   input loads overlap iteration i's attention/tail in the repeat loop.
"""

import os
import numpy as np

S = 2048
D = 1024
H = 16
DH = 64
NCORES = 8

_PROG_CACHE = {}


def _build(bounds, reps=1, debug=False):
    import contextlib

    import concourse.bacc as bacc
    import concourse.mybir as mybir
    import concourse.tile as tile
    from concourse.bass import ts

    f32 = mybir.dt.float32
    bf16 = mybir.dt.bfloat16
    AF = mybir.ActivationFunctionType

    segs = [(bounds[g], bounds[g + 1]) for g in range(4) if bounds[g + 1] > bounds[g]]

    allwins = []   # (w0, w1, anchor): window, plus 128-wide transpose anchor
    seg_wins = []
    for (lo, hi) in segs:
        idxs = []
        for w0 in range(lo, hi, 128):
            w1 = min(w0 + 128, hi)
            idxs.append(len(allwins))
            allwins.append((w0, w1, min(w0, S - 128)))
        seg_wins.append(idxs)
    NW = len(allwins)

    # chunks that must be projected before seg si's attention: its qa/ka
    # columns AND every window's 128-wide transpose slice (else the vga
    # drain would be emitted after its readers, which Tile treats as WAR,
    # not RAW - the cause of a first-run-only garbage bug)
    seg_need = []
    for si, (lo, hi) in enumerate(segs):
        need = (hi - 1) // 512
        for widx in seg_wins[si]:
            need = max(need, (allwins[widx][2] + 127) // 512)
        seg_need.append(min(3, need))

    # per-seg balanced score chunks of <=512 columns (the last segment is
    # split at least in two so its tail tiles can start earlier)
    def _chunks(lo, hi, nmin=1):
        n = max(-(-(hi - lo) // 512), nmin)
        base, rem = (hi - lo) // n, (hi - lo) % n
        out, p = [], lo
        for i in range(n):
            w = base + (1 if i < rem else 0)
            out.append((p, p + w))
            p += w
        return out
    seg_sc = [_chunks(lo, hi, 2 if si == len(segs) - 1 else 1)
              for si, (lo, hi) in enumerate(segs)]

    nc = bacc.Bacc(None, target_bir_lowering=False)

    xts_d = nc.dram_tensor("xts", [128, 8 * S], bf16, kind="ExternalInput")
    wqs_d = nc.dram_tensor("wqs", [128, 1024], bf16, kind="ExternalInput")
    wks_d = nc.dram_tensor("wks", [128, 1024], bf16, kind="ExternalInput")
    wvs_d = nc.dram_tensor("wvs", [128, 1024], bf16, kind="ExternalInput")
    wos_d = nc.dram_tensor("wos", [128, 1024], bf16, kind="ExternalInput")
    cs_d = nc.dram_tensor("cs2", [128, S], bf16, kind="ExternalInput")
    sn_d = nc.dram_tensor("sn2", [128, S], bf16, kind="ExternalInput")
    sel_d = nc.dram_tensor("sels", [128, 256], bf16, kind="ExternalInput")
    id_d = nc.dram_tensor("idon", [128, 128], bf16, kind="ExternalInput")
    out_d = nc.dram_tensor("outp", [S, D], bf16, kind="ExternalOutput")
    if debug:
        dbg = {n: nc.dram_tensor(f"dbg_{n}", shp, bf16, kind="ExternalOutput")
               for n, shp in (("qa", [128, S]), ("ka", [128, S]),
                              ("vt", [128, S]), ("osb", [128, S]))}
        nwn = len(allwins)
        dbg["vga"] = nc.dram_tensor("dbg_vga", [128, nwn * 130], bf16,
                                    kind="ExternalOutput")

    with tile.TileContext(nc, linearize=bool(os.environ.get("KLIN"))) as tc:
        rep_ctx = (tc.For_i(0, reps, 1, hint_engines=(
            mybir.EngineType.PE, mybir.EngineType.DVE,
            mybir.EngineType.Activation, mybir.EngineType.SP,
            mybir.EngineType.Pool))
                   if reps > 1 else contextlib.nullcontext())
        with rep_ctx, \
             tc.tile_pool(name="pj", bufs=1, space="PSUM") as PJ, \
             tc.tile_pool(name="stp", bufs=3, space="PSUM") as STP, \
             tc.tile_pool(name="pop", bufs=3, space="PSUM") as POP, \
             tc.tile_pool(name="wp", bufs=1) as wp, \
             tc.tile_pool(name="xp", bufs=1) as xp, \
             tc.tile_pool(name="pers", bufs=1) as pers, \
             tc.tile_pool(name="abp", bufs=3) as abp, \
             tc.tile_pool(name="up", bufs=4) as up, \
             tc.tile_pool(name="ocp", bufs=4) as ocp:

            wq_sb = wp.tile([128, 1024], bf16)
            wk_sb = wp.tile([128, 1024], bf16)
            wv_sb = wp.tile([128, 1024], bf16)
            wo0_sb = wp.tile([64, 1024], bf16)
            wo1_sb = wp.tile([64, 1024], bf16)
            cs_sb = wp.tile([128, S], bf16)
            sn_sb = wp.tile([128, S], bf16)
            sel_sb = wp.tile([128, 256], bf16)
            id_sb = wp.tile([128, 128], bf16)
            for t, dt_ in ((wq_sb, wqs_d), (wk_sb, wks_d), (wv_sb, wvs_d),
                           (cs_sb, cs_d), (sn_sb, sn_d), (sel_sb, sel_d),
                           (id_sb, id_d)):
                nc.sync.dma_start(t[:], dt_[:])
            nc.sync.dma_start(wo0_sb[:], wos_d[0:64, :])
            nc.sync.dma_start(wo1_sb[:], wos_d[64:128, :])
            xt = [xp.tile([128, S], bf16, name=f"xt{d}") for d in range(8)]
            for half in range(2):
                for d in range(8):
                    h0 = half * 1024
                    nc.sync.dma_start(
                        xt[d][:, h0:h0 + 1024],
                        xts_d[:, d * S + h0:d * S + h0 + 1024])

            qa = pers.tile([128, S], bf16, name="qa")
            ka = pers.tile([128, S], bf16, name="ka")
            vt_sb = pers.tile([128, S], bf16, name="vt")
            vga = pers.tile([128, NW, 130], bf16, name="vga")
            osb0 = pers.tile([64, S], bf16, name="osb0")
            osb1 = pers.tile([64, S], bf16, name="osb1")
            rvs = [pers.tile([65, S], bf16, name=f"rv{h}") for h in (0, 1)]
            ones_sb = pers.tile([65, 64], bf16, name="ones")

            # ones column 64 of each head's 65-block in vga; ones rows for
            # the 1/r broadcast matmul
            ones_cols = vga.rearrange(
                "p w (a c) -> p w a c", a=2, c=65)[:, :, :, 64:65]
            nc.gpsimd.memset(ones_cols.bitcast(mybir.dt.uint16), 0x3F80)
            nc.gpsimd.memset(ones_sb[:].bitcast(mybir.dt.uint16), 0x3F80)

            npj = [0]

            def pj_tile(name):
                t = PJ.tile([128, 512], f32, tag=f"pj{npj[0] % 2}", name=name)
                npj[0] += 1
                return t

            def proj_chunk(ci):
                c0 = ci * 512
                qb = pj_tile(f"pq{ci}")
                for d in range(8):
                    nc.tensor.matmul(qb[:], wq_sb[:, ts(d, 128)],
                                     xt[d][:, c0:c0 + 512],
                                     start=(d == 0), stop=(d == 7))
                aq = abp.tile([128, 512], bf16, tag="aq", name=f"aq{ci}")
                bq = abp.tile([128, 512], bf16, tag="bq", name=f"bq{ci}")
                nc.vector.tensor_mul(aq[:], qb[:], cs_sb[:, c0:c0 + 512])
                nc.vector.tensor_mul(bq[:], qb[:], sn_sb[:, c0:c0 + 512])
                kb = pj_tile(f"pk{ci}")
                for d in range(8):
                    nc.tensor.matmul(kb[:], wk_sb[:, ts(d, 128)],
                                     xt[d][:, c0:c0 + 512],
                                     start=(d == 0), stop=(d == 7))
                ak = abp.tile([128, 512], bf16, tag="ak", name=f"ak{ci}")
                bk = abp.tile([128, 512], bf16, tag="bk", name=f"bk{ci}")
                nc.vector.tensor_mul(ak[:], kb[:], cs_sb[:, c0:c0 + 512])
                nc.vector.tensor_mul(bk[:], kb[:], sn_sb[:, c0:c0 + 512])
                rq = pj_tile(f"rq{ci}")
                nc.tensor.matmul(rq[:], sel_sb[:, 128:256], bq[:],
                                 start=True, stop=False)
                nc.tensor.matmul(rq[:], sel_sb[:, 0:128], aq[:],
                                 start=False, stop=True)
                nc.scalar.copy(qa[:, c0:c0 + 512], rq[:])
                rk = pj_tile(f"rk{ci}")
                nc.tensor.matmul(rk[:], sel_sb[:, 128:256], bk[:],
                                 start=True, stop=False)
                nc.tensor.matmul(rk[:], sel_sb[:, 0:128], ak[:],
                                 start=False, stop=True)
                nc.scalar.copy(ka[:, c0:c0 + 512], rk[:])
                vb = pj_tile(f"pv{ci}")
                for d in range(8):
                    nc.tensor.matmul(vb[:], wv_sb[:, ts(d, 128)],
                                     xt[d][:, c0:c0 + 512],
                                     start=(d == 0), stop=(d == 7))
                nc.scalar.copy(vt_sb[:, c0:c0 + 512], vb[:])

            def vt_window(widx):
                w0, w1, anchor = allwins[widx]
                wd = w1 - w0
                poff = w0 - anchor
                tp = STP.tile([128, 128], bf16, tag="st", name=f"tp{widx}")
                nc.tensor.transpose(tp[:], vt_sb[:, anchor:anchor + 128],
                                    id_sb[:])
                dst = vga[poff:poff + wd, widx, :].rearrange(
                    "p (a c) -> p a c", a=2, c=65)[:, :, 0:64]
                src = tp[poff:poff + wd, :].rearrange(
                    "p (a c) -> p a c", a=2, c=64)
                nc.vector.tensor_copy(dst, src)

            def attention_chunk(si, clo, chi):
                wins = seg_wins[si]
                cw = chi - clo
                plo, phi = clo, chi
                if cw % 2:
                    # pad left when possible: those columns are always
                    # already written (earlier chunks), never read-ahead
                    if clo > 0:
                        plo -= 1
                    else:
                        phi += 1
                cwp = phi - plo
                off = clo - plo

                for h in (0, 1):
                    pot = POP.tile([65, 512], f32, tag="po",
                                   name=f"po{si}_{h}")

                    def _av(ti, widx, u):
                        w0, w1, anchor = allwins[widx]
                        wd = w1 - w0
                        poff = w0 - anchor
                        nc.tensor.matmul(
                            pot[:, 0:cwp],
                            vga[poff:poff + wd, widx, ts(h, 65)],
                            u[0:wd, 0:cwp],
                            start=(ti == 0), stop=(ti == len(wins) - 1),
                            tile_position=(0, 0) if poff else None)

                    pend = None
                    for ti, widx in enumerate(wins):
                        w0, w1, anchor = allwins[widx]
                        wd = w1 - w0
                        stl = STP.tile([128, 512], f32, tag="st",
                                       name=f"st{si}_{h}")
                        nc.tensor.matmul(stl[0:wd, 0:cwp],
                                         ka[ts(h, 64), w0:w1],
                                         qa[ts(h, 64), plo:phi],
                                         start=True, stop=True)
                        if pend is not None:
                            _av(*pend)
                        u = up.tile([128, 512], bf16, tag="u", name="u")
                        nc.scalar.activation(u[0:wd, 0:cwp],
                                             stl[0:wd, 0:cwp], AF.Exp)
                        pend = (ti, widx, u)
                    _av(*pend)

                    # renorm: broadcast the fused denominator row across 64
                    # partitions with a C=1 matmul, take 1/r while draining
                    # it out of psum, and apply during the pot drain
                    nc.vector.tensor_copy(rvs[h][64:65, plo:phi],
                                          pot[64:65, 0:cwp])
                    rtv = STP.tile([64, 512], f32, tag="st", name="rtv")
                    nc.tensor.matmul(rtv[:, 0:cwp], ones_sb[64:65, :],
                                     rvs[h][64:65, plo:phi],
                                     start=True, stop=True)
                    rv = up.tile([64, 512], f32, tag="rv", name="rv")
                    nc.vector.reciprocal_approx_fast(rv[:, 0:cwp],
                                                     rtv[:, 0:cwp])
                    dst = osb0 if h == 0 else osb1
                    nc.vector.tensor_mul(dst[:, clo:chi],
                                         pot[0:64, off:off + cw],
                                         rv[:, off:off + cw])

            noc = [0]

            def tail_tile(i):
                oc = ocp.tile([128, 1024], bf16, tag="oc", name=f"oc{i}")
                for jj in (0, 1):
                    po = pj_tile(f"po{i}_{jj}")
                    nc.tensor.matmul(po[:], osb0[:, ts(i, 128)],
                                     wo0_sb[:, ts(jj, 512)],
                                     start=True, stop=False)
                    nc.tensor.matmul(po[:], osb1[:, ts(i, 128)],
                                     wo1_sb[:, ts(jj, 512)],
                                     start=False, stop=True)
                    if noc[0] % 2:
                        nc.scalar.copy(oc[:, ts(jj, 512)], po[:])
                    else:
                        nc.vector.tensor_copy(oc[:, ts(jj, 512)], po[:])
                    noc[0] += 1
                # out DMAs ride the gpsimd SWDGE queue: nothing on-chip
                # reads them, and sync/scalar queues stay unblocked
                nc.gpsimd.dma_start(out_d[ts(i, 128), :], oc[:])

            emitted_c = 0
            done_t = 0
            done = 0
            pending_lim = [0]
            for si, (lo, hi) in enumerate(segs):
                ci_need = seg_need[si]
                while emitted_c <= ci_need:
                    proj_chunk(emitted_c)
                    emitted_c += 1
                    cov = emitted_c * 512
                    while done_t < NW and allwins[done_t][2] + 128 <= cov:
                        vt_window(done_t)
                        done_t += 1
                last = si == len(segs) - 1
                for (clo, chi) in seg_sc[si]:
                    # lagged tails: emit tiles made ready by the PREVIOUS
                    # chunk now, so they queue behind this chunk's matmuls
                    # instead of stalling PE on the renorm chain
                    while done < pending_lim[0]:
                        tail_tile(done)
                        done += 1
                    attention_chunk(si, clo, chi)
                    pending_lim[0] = 16 if (last and chi == hi) else chi // 128
            while done < pending_lim[0]:
                tail_tile(done)
                done += 1

            if debug:
                for n, t in (("qa", qa), ("ka", ka), ("vt", vt_sb)):
                    nc.sync.dma_start(dbg[n][:], t[:])
                nc.sync.dma_start(dbg["osb"][0:64, :], osb0[:])
                nc.sync.dma_start(dbg["osb"][64:128, :], osb1[:])
                nc.sync.dma_start(
                    dbg["vga"][:],
                    vga.rearrange("p w c -> p (w c)"))

    nc.compile()
    return nc


def _host_tensors(x, seg, fc, fs, wq, wk, wv, wo):
    import ml_dtypes
    bf16 = ml_dtypes.bfloat16

    c64 = np.repeat(fc.T, 2, axis=0)
    s64 = np.empty((64, S), np.float32)
    s64[0::2] = fs.T
    s64[1::2] = -fs.T
    cos2 = np.ascontiguousarray(np.tile(c64, (2, 1))).astype(bf16)
    sin2 = np.ascontiguousarray(np.tile(s64, (2, 1))).astype(bf16)

    sel = np.zeros((128, 256), np.float32)
    sel[np.arange(128), np.arange(128)] = 1.0           # isel = I
    sel[np.arange(128) ^ 1, 128 + np.arange(128)] = 1.0  # psel[p^1, p]
    sels = sel.astype(bf16)

    idon = np.eye(128, dtype=np.float32).astype(bf16)

    xts = np.ascontiguousarray(
        x.T.reshape(8, 128, S).transpose(1, 0, 2)).reshape(128, 8 * S)
    xts = xts.astype(bf16)

    def wstack(w, scale):
        out = []
        for m in range(NCORES):
            wl = (w[m * 128:(m + 1) * 128, :] * scale).T.astype(np.float32)
            out.append(np.ascontiguousarray(
                wl.reshape(8, 128, 128).transpose(1, 0, 2)).reshape(
                    128, 1024).astype(bf16))
        return out

    wqs = wstack(wq, 0.125)
    wks = wstack(wk, 1.0)
    wvs = wstack(wv, 1.0)
    wos = [np.ascontiguousarray(wo[:, m * 128:(m + 1) * 128].T).astype(bf16)
           for m in range(NCORES)]

    common = {"xts": xts, "cs2": cos2, "sn2": sin2, "sels": sels,
              "idon": idon}
    in_maps = []
    for m in range(NCORES):
        im = dict(common)
        im["wqs"] = wqs[m]
        im["wks"] = wks[m]
        im["wvs"] = wvs[m]
        im["wos"] = wos[m]
        in_maps.append(im)
    return in_maps


def kernel(x, seg_ids, freqs_cos, freqs_sin, wq, wk, wv, wo):
    x = np.asarray(x, np.float32).reshape(S, D)
    seg = np.asarray(seg_ids).astype(np.int64)
    fc = np.asarray(freqs_cos, np.float32)
    fs = np.asarray(freqs_sin, np.float32)
    wq = np.asarray(wq, np.float32)
    wk = np.asarray(wk, np.float32)
    wv = np.asarray(wv, np.float32)
    wo = np.asarray(wo, np.float32)

    bounds = tuple(int(b) for b in np.searchsorted(seg, np.arange(5)))
    if bounds not in _PROG_CACHE:
        _PROG_CACHE[bounds] = _build(bounds)
    nc = _PROG_CACHE[bounds]

    in_maps = _host_tensors(x, seg, fc, fs, wq, wk, wv, wo)

    from concourse.bass_utils import run_bass_kernel_spmd

    trace = bool(os.environ.get("BASS_KERNEL_TRACE"))
    res = run_bass_kernel_spmd(nc, in_maps, core_ids=list(range(NCORES)),
                               trace=trace)
    if trace and res.exec_time_ns is not None:
        print(f"HW exec time: {res.exec_time_ns} ns")
        if res.instructions_and_trace is not None:
            print("trace:", res.instructions_and_trace[1])

    out = np.zeros((S, D), np.float32)
    for r in res.results:
        out += np.asarray(r["outp"], dtype=np.float32)
    return out.reshape(1, S, D)
